# revision 43
# baseline (speedup 1.0000x reference)
"""Causal self-attention (B=4, T=2048, C=1024, 16 heads) on 8 trn2 NeuronCores.

Sharding: core c handles batch b = c//2 and head-group hg = c%2 (8 of 16 heads).
Each core computes QKV projection for its heads, causal attention, and a partial
output projection (row-sharded W_proj); the host sums the two partials per batch
and adds b_proj.

Device layout notes:
 - x is fed pre-transposed ([C, T]) so the contraction dim C lands on SBUF
   partitions with no on-device transpose.
 - Scores are computed transposed (S^T[k, q]) so softmax's reduction over k can
   be done by the PE via a ones-column appended to V (row k of S^T is a
   partition; summing over partitions is a matmul).
 - Softmax skips the max-subtraction: scores/8 are ~N(0,1) here, exp is safe in
   fp32 and the result is mathematically identical.
 - All matmul operands are fp16 (fp32 PSUM accumulate): same PE stream rate as
   fp32, but FWL (fast weight load) halves LDWEIGHTS time, and SBUF/DMA
   traffic halves. fp16's 11-bit mantissa keeps end-to-end rel err ~3e-3.

Performance structure (v14, ~298us vs the 405us v4 baseline), built around
three engine limits measured in traces: PE matmul streaming (~213ns per
N=512), per-matmul LDWEIGHTS serialization (fp16 FWL halves it), and the ACT
engine's exp cost ((N+352)/1.2 ns per instruction):
 - Scores matmuls have K=64 (head dim): the two heads of a feature-pair (fq)
   live on partitions 0-63 / 64-127, so their score MMs target disjoint PE
   row-groups (tile_position auto-derived from base_partition) and run
   CONCURRENTLY when issued back-to-back - halving scores PE time.
 - Attention is organized in pair-units (half, fq, qc, kt) where qc is a
   512-token q chunk: one PSUM tile [128, 1024] holds both heads' scores
   (A: cols 0-511, B: 512-1023), so ONE exp instruction covers two heads
   (fewer ACT fixed overheads). Unwritten diag-trim columns are exp'd as
   garbage but never streamed into the y matmuls.
 - y accumulates per pair-group into a [65, 1024] PSUM region (rows 0-63 y,
   row 64 rowsum via the V ones-column; A cols 0-511, B 512-1023).
 - Normalization per pair-group: rowsum row AND unnormalized y rows copied
   to SBUF first (two plain-shape DVE copies: py's PSUM slot frees after
   ~2.3us instead of the full ~5.5us chain; a single [65,1024] copy produced
   NaN columns on HW - keep the two-copy form), then reciprocal_approx_fast
   (DVE; its bit-trick seed misreads PSUM), partition_broadcast (GpSimd;
   input must be a partition-0 tile, GpSimd cannot read PSUM), and two DVE
   multiplies -> yn (fp16).
 - Phase 1a (QKV for tokens 0..1023): chunk-outer loop over 8 PSUM banks so
   the first matmul starts right after the first w/x chunk DMA lands; range 1
   walks banks in reverse (they free in reverse order); v copies run on the
   otherwise-idle ACT engine (which, unlike GpSimd, can read PSUM).
 - Half-0's attention stream is PE-bound: QKV for tokens 1024..2047 is
   interleaved as whole fillers at pair-group boundaries. Half-1's stream is
   ACT-paced: exp(i+1) is issued before y(i) so ACT always has one queued
   exp, and filler/projection work drips in <=~1us chunks (one per unit, two
   ahead of a group-start y) so no PE burst outruns that 1-exp lookahead.
 - Output projection: half-0's tiles and half-1's qc0 tiles drip into half
   1's stream (chunked, through the psy pool); half-1's qc1 tiles are the
   tail. Output partials are fp16, summed in fp32 on the host. Output DMA
   overlaps compute. The dense PE queue also keeps the HAM clock at 2.4GHz.

HW exec time varies run-to-run (~0.5% warm; occasionally ~1.2x when the chip
enters the P0 power-state downclock) - compare kernels by best-of-3.
"""
import numpy as np

T = 2048          # tokens per batch element
C = 1024          # embed dim
H = 8             # heads per core
D = 64            # head dim
CC = 8            # contraction chunks (C / 128)

_CACHE = {}


def _build_nc():
    from concourse import bacc
    import concourse.mybir as mybir
    import concourse.tile as tile

    f32 = mybir.dt.float32
    f16 = mybir.dt.float16
    bf16 = mybir.dt.bfloat16
    EXP = mybir.ActivationFunctionType.Exp

    nc = bacc.Bacc("TRN2", num_devices=8, debug=False)

    xt_d = nc.dram_tensor("xt", [C, T], f16, kind="ExternalInput")
    wqkv_d = nc.dram_tensor("wqkv", [C, 1536], f16, kind="ExternalInput")
    bqk_d = nc.dram_tensor("bqk", [128, 8], f32, kind="ExternalInput")
    bv_d = nc.dram_tensor("bv", [1, 512], f16, kind="ExternalInput")
    wproj_d = nc.dram_tensor("wproj", [512, C], f16, kind="ExternalInput")
    ones_d = nc.dram_tensor("ones", [1, 128], f16, kind="ExternalInput")
    maskb_d = nc.dram_tensor("maskb", [128, 256], bf16, kind="ExternalInput")
    out_d = nc.dram_tensor("out", [T, C], f16, kind="ExternalOutput")

    with tile.TileContext(nc) as tc:
      with tc.tile_pool(name="persist", bufs=1) as pp:
        # persistent SBUF: qk^T [1024 feats, T] f16, v [T, 8*(64+1)] f16
        qk_sb = [pp.tile([128, T], f16, tag=f"qk{f}", name=f"qk{f}") for f in range(8)]
        v_sb = [pp.tile([128, H * 65], f16, tag=f"v{t}", name=f"v{t}") for t in range(16)]
        wp_sb = [pp.tile([128, C], f16, tag=f"wp{i}", name=f"wp{i}") for i in range(4)]
        maskb_sb = pp.tile([128, 256], bf16, tag="maskb")
        ones_sb = pp.tile([1, 128], f16, tag="ones")
        bqk_sb = pp.tile([128, 8], f32, tag="bqk")
        bv_sb = pp.tile([1, 512], f16, tag="bv")

        def persist_dmas():
            # issued AFTER the first w/x chunk DMAs: nothing here is needed
            # until attention / projection, so keep it off the critical path
            nc.sync.dma_start(maskb_sb[:], maskb_d[:])
            nc.sync.dma_start(bqk_sb[:], bqk_d[:])
            for i in range(4):
                nc.sync.dma_start(wp_sb[i][:], wproj_d[i * 128:(i + 1) * 128, :])

        for t in range(16):
            # ones column at position 64 of each head's 65-wide V block
            nc.gpsimd.memset(
                v_sb[t][:].rearrange("p (h e) -> p h e", e=65)[:, :, 64:65], 1.0
            )

        # Filler work (QKV for tokens 1024..2047, and the output projection)
        # is emitted as CHUNK LISTS: closures each costing <=~1us of PE time,
        # dripped one-per-attention-unit so no single PE burst outruns the
        # 1-exp ACT lookahead (which would stall the ACT-paced pipeline).
        def qk_feature_chunks(f, xs2, dst):
            # q/k features f*128..f*128+128 for tokens dst..dst+1024
            st = {}

            def mm_chunk(h, cr):
                def c_(pool):
                    if h == 0 and cr == 0:
                        st["pq"] = pool.tile([128, 1024], f32, tag="py", name="pq")
                    for c in range(cr, cr + 4):
                        nc.tensor.matmul(
                            st["pq"][:, h * 512:(h + 1) * 512],
                            w_sb[c][:, f * 128:(f + 1) * 128],
                            xs2[h][c][:],
                            start=(c == 0), stop=(c == CC - 1),
                        )
                return c_

            def c_add(pool):
                nc.vector.tensor_scalar_add(
                    qk_sb[f][:, dst:dst + 1024], st["pq"][:], bqk_sb[:, f:f + 1]
                )

            return [mm_chunk(0, 0), mm_chunk(0, 4),
                    mm_chunk(1, 0), mm_chunk(1, 4), c_add]

        def v_tile_chunks(tl, xs2, tg, act_copy=False):
            # v for 128 tokens (tl-th 128-block of xs2) -> v_sb[tg]
            xs = xs2[tl // 4]
            t0 = (tl % 4) * 128
            st = {}

            def c0(pool):
                st["pv"] = pool.tile([128, 1024], f32, tag="py", name="pv")
                for c in range(4):
                    nc.tensor.matmul(
                        st["pv"][:, 0:512], xs[c][:, t0:t0 + 128],
                        w_sb[c][:, 1024:1536],
                        start=(c == 0), stop=False,
                    )

            def c1(pool):
                for c in range(4, CC):
                    nc.tensor.matmul(
                        st["pv"][:, 0:512], xs[c][:, t0:t0 + 128],
                        w_sb[c][:, 1024:1536],
                        start=False, stop=False,
                    )
                nc.tensor.matmul(st["pv"][:, 0:512], ones_sb[:], bv_sb[:],
                                 start=False, stop=True)

            def c2(pool):
                # in stream 0, DVE runs hot (norm chains + bias adds) while
                # ACT has slack and can read PSUM -> copy on ACT there; in
                # the ACT-bound stream 1, keep the copy on DVE
                if act_copy:
                    nc.scalar.copy(
                        v_sb[tg][:].rearrange("p (h e) -> p h e", e=65)[:, :, 0:64],
                        st["pv"][:, 0:512].rearrange("p (h e) -> p h e", e=64),
                    )
                else:
                    nc.vector.tensor_copy(
                        v_sb[tg][:].rearrange("p (h e) -> p h e", e=65)[:, :, 0:64],
                        st["pv"][:, 0:512].rearrange("p (h e) -> p h e", e=64),
                    )

            return [c0, c1, c2]

        # ---------------- Phase 1a: QKV for tokens 0..1023 (ranges 0,1) -----
        with (
            tc.tile_pool(name="ynp", bufs=2) as ynp,
            tc.tile_pool(name="epool", bufs=4) as ep,
            tc.tile_pool(name="rpool", bufs=2) as rp,
        ):
          yn_cur = {}
          py_cur = {}
          proj_q = []
          with (
            tc.tile_pool(name="w", bufs=1) as pw,
            tc.tile_pool(name="xa", bufs=2) as pxa,
          ):
            w_sb = [pw.tile([128, 1536], f16, tag=f"w{c}", name=f"w{c}") for c in range(CC)]
            with (
                tc.tile_pool(name="psA", bufs=1, space="PSUM") as psA,
            ):
                x_r = {}
                for c in range(CC):
                    # pair chunk DMAs so the first matmul group starts early
                    if c < 2:
                        # split the first chunks across DMA queues so the
                        # first matmul group starts ~2us earlier
                        nc.sync.dma_start(w_sb[c][:, 0:768],
                                          wqkv_d[c * 128:(c + 1) * 128, 0:768])
                        nc.sync.dma_start(w_sb[c][:, 768:1536],
                                          wqkv_d[c * 128:(c + 1) * 128, 768:1536])
                    else:
                        nc.sync.dma_start(w_sb[c][:],
                                          wqkv_d[c * 128:(c + 1) * 128, :])
                    t_ = pxa.tile([128, 512], f16, tag=f"x{c}", name=f"x{c}")
                    nc.sync.dma_start(t_[:], xt_d[c * 128:(c + 1) * 128, 0:512])
                    x_r.setdefault(0, []).append(t_)
                    if c == 0:
                        # tiny, needed a few us in by the v-tile bias matmul
                        nc.sync.dma_start(ones_sb[:], ones_d[:])
                        nc.sync.dma_start(bv_sb[:], bv_d[:])
                    if c == CC - 1:
                        persist_dmas()
                for r in (0, 1):
                    if r == 1:
                        x_r[1] = []
                        for c in range(CC):
                            t_ = pxa.tile([128, 512], f16, tag=f"x{c}", name=f"x{c}")
                            nc.sync.dma_start(
                                t_[:], xt_d[c * 128:(c + 1) * 128, 512:1024])
                            x_r[1].append(t_)
                    # chunk-outer over 8 psum banks: chunk c usable on
                    # arrival. Range 1 walks features in reverse so it starts
                    # on the banks range 0 freed first (qk adds finish before
                    # the v copies on banks 0-3).
                    forder = list(range(8)) if r == 0 else list(range(7, -1, -1))
                    tlorder = list(range(4)) if r == 0 else list(range(3, -1, -1))
                    pq8 = {f: psA.tile([128, 512], f32, tag=f"b{f}", name=f"b{f}")
                           for f in forder}
                    for c in range(CC):
                        for f in forder:
                            nc.tensor.matmul(
                                pq8[f][:], w_sb[c][:, f * 128:(f + 1) * 128],
                                x_r[r][c][:],
                                start=(c == 0), stop=(c == CC - 1),
                            )
                    for f in forder:
                        nc.vector.tensor_scalar_add(
                            qk_sb[f][:, r * 512:(r + 1) * 512], pq8[f][:],
                            bqk_sb[:, f:f + 1],
                        )
                    for tl in tlorder:
                        tg = r * 4 + tl
                        pv = psA.tile([128, 512], f32, tag=f"b{tl}", name=f"pv{tl}")
                        for c in range(CC):
                            nc.tensor.matmul(
                                pv[:], x_r[r][c][:, tl * 128:(tl + 1) * 128],
                                w_sb[c][:, 1024:1536],
                                start=(c == 0), stop=False,
                            )
                        nc.tensor.matmul(pv[:], ones_sb[:], bv_sb[:],
                                         start=False, stop=True)
                        # ACT is idle in phase 1a and can read PSUM: v copies
                        # go there so the DVE only carries the qk bias-adds
                        nc.scalar.copy(
                            v_sb[tg][:].rearrange("p (h e) -> p h e", e=65)[:, :, 0:64],
                            pv[:].rearrange("p (h e) -> p h e", e=64),
                        )

            # x for tokens 1024..2047 (ranges 2,3), used by the interleaved
            # QKV: two more generations of the xa pool's chunk tiles
            x23 = []
            for h, lo in enumerate((1024, 1536)):
                xs = []
                for c in range(CC):
                    t_ = pxa.tile([128, 512], f16, tag=f"x{c}", name=f"x{c}")
                    nc.sync.dma_start(t_[:], xt_d[c * 128:(c + 1) * 128, lo:lo + 512])
                    xs.append(t_)
                x23.append(xs)

            # ---------------- Phase 2: attention ----------------------------
            # Pair-units (half, fq, qc, kt): heads A=2fq (partitions 0-63)
            # and B=2fq+1 (64-127) computed together; their score MMs hit
            # disjoint PE row-groups and run concurrently.
            if True:
                def emit_scores(pss, u):
                    half, fq, qc, kt = u
                    qg = 1024 * half + 512 * qc
                    a = max(0, kt * 128 - qg)
                    diag = kt >= 8 * half + 4 * qc
                    kcol = slice(kt * 128, (kt + 1) * 128)
                    qcol = slice(qg + a, qg + 512)
                    ps = pss.tile([128, 1024], f32, tag="s", name="ps_s")
                    nc.tensor.matmul(
                        ps[0:128, a:512],
                        qk_sb[4 + fq][0:64, kcol], qk_sb[fq][0:64, qcol],
                        start=True, stop=not diag,
                    )
                    nc.tensor.matmul(
                        ps[0:128, 512 + a:1024],
                        qk_sb[4 + fq][64:128, kcol], qk_sb[fq][64:128, qcol],
                        start=True, stop=not diag,
                    )
                    if diag:
                        # += -1e30 * upper_strict on each head's diag block
                        nc.tensor.matmul(
                            ps[:, a:a + 128],
                            maskb_sb[:, 0:128], maskb_sb[:, 128:256],
                            start=False, stop=True,
                        )
                        nc.tensor.matmul(
                            ps[:, 512 + a:512 + a + 128],
                            maskb_sb[:, 0:128], maskb_sb[:, 128:256],
                            start=False, stop=True,
                        )
                    return ps

                def emit_exp(u, ps):
                    half, fq, qc, kt = u
                    qg = 1024 * half + 512 * qc
                    a = max(0, kt * 128 - qg)
                    # one instruction covers both heads' valid regions
                    # ([a,512) and [512+a,1024)); cols [512-a,512) of e are
                    # exp(stale PSUM) and are never streamed into y.
                    e = ep.tile([128, 1024], f16, tag="e", name="e_t")
                    nc.scalar.activation(
                        e[:, 0:1024 - a], ps[:, a:1024], EXP, scale=0.125,
                    )
                    return e

                def emit_y(psy, u, e):
                    half, fq, qc, kt = u
                    qg = 1024 * half + 512 * qc
                    a = max(0, kt * 128 - qg)
                    last = 8 * half + 4 * qc + 3
                    if kt == 0:
                        py_cur[(half, fq, qc)] = psy.tile(
                            [128, 1024], f32, tag="py", name="py_t")
                    py = py_cur[(half, fq, qc)]
                    nc.tensor.matmul(
                        py[0:65, a:512],
                        v_sb[kt][:, (2 * fq) * 65:(2 * fq + 1) * 65],
                        e[:, 0:512 - a],
                        start=(kt == 0), stop=(kt == last),
                    )
                    nc.tensor.matmul(
                        py[0:65, 512 + a:1024],
                        v_sb[kt][:, (2 * fq + 1) * 65:(2 * fq + 2) * 65],
                        e[:, 512:1024 - a],
                        start=(kt == 0), stop=(kt == last),
                    )

                def emit_norm(u):
                    half, fq, qc, _ = u
                    py = py_cur.pop((half, fq, qc))
                    # ONE copy of py (y rows + rowsum row) to SBUF frees the
                    # PSUM slot after a single ~1.3us DVE op; the broadcast /
                    # reciprocal / normalize run lazily from the SBUF copy
                    # (the custom-DVE recip's bit-trick seed misreads PSUM,
                    # and GpSimd cannot read PSUM at all).
                    # stage rowsum AND unnormalized y to SBUF up front: py's
                    # PSUM slot frees after these two copies (~2.3us) instead
                    # of after the full recip/broadcast/mul chain (~5.5us),
                    # so the next group's y never waits on this group's norm
                    rs = rp.tile([1, 1024], f32, tag="rs", name="rs_t")
                    nc.vector.tensor_copy(rs[:], py[64:65, 0:1024])
                    ya = rp.tile([64, 1024], f32, tag="ya", name="ya_t")
                    nc.vector.tensor_copy(ya[:], py[0:64, 0:1024])
                    r = rp.tile([1, 1024], f32, tag="r", name="r_t")
                    nc.vector.reciprocal_approx_fast(r[:], rs[:])
                    rb = rp.tile([64, 1024], f32, tag="rb", name="rb_t")
                    nc.gpsimd.partition_broadcast(rb[:], r[:])
                    qcc = slice(qc * 512, qc * 512 + 512)
                    nc.vector.tensor_mul(
                        yn_cur[half][fq][0:64, qcc], ya[:, 0:512], rb[:, 0:512],
                    )
                    nc.vector.tensor_mul(
                        yn_cur[half][fq][64:128, qcc], ya[:, 512:1024],
                        rb[:, 512:1024],
                    )
                    if half == 1 and fq == 3:
                        # qc-outer ordering: all 4 fq done for this qc ->
                        # the qc's token blocks can be projected already
                        for tt in range(qc * 4, qc * 4 + 4):
                            proj_q.append((half, tt))
                    elif half == 0 and fq == 3 and qc == 1:
                        for tt in range(8):
                            proj_q.append((half, tt))

                def proj_chunks(item, obp, split=False):
                    half, tt = item
                    st = {}

                    def mm_chunk(fcs):
                        def c_(pool):
                            if fcs[0] == 0:
                                st["po"] = pool.tile(
                                    [128, 1024], f32, tag="py", name="po_t")
                            for fc in fcs:
                                for n in range(2):
                                    nc.tensor.matmul(
                                        st["po"][:, n * 512:(n + 1) * 512],
                                        yn_cur[half][fc][:, tt * 128:(tt + 1) * 128],
                                        wp_sb[fc][:, n * 512:(n + 1) * 512],
                                        start=(fc == 0), stop=(fc == 3),
                                    )
                        return c_

                    def c_out(pool):
                        ob = obp.tile([128, C], f16, tag="ob")
                        nc.vector.tensor_copy(ob[:], st["po"][:])
                        nc.sync.dma_start(
                            out_d[half * 1024 + tt * 128:
                                  half * 1024 + (tt + 1) * 128, :],
                            ob[:],
                        )

                    if split:
                        # tail form: fc3 (which waits the final norm) in its
                        # own chunk so fc0-2 work can bridge the norm latency
                        return [mm_chunk((0, 1)), mm_chunk((2,)),
                                mm_chunk((3,)), c_out]
                    return [mm_chunk((0, 1)), mm_chunk((2, 3)), c_out]

                def run_stream(pss, psy, units, fillers, obp=None,
                               chunked=False):
                    # scores(i+1) AND exp(i+1) are traced before y(i): the
                    # queued exp keeps ACT fed through PE bursts.
                    # chunked=False (PE-bound stream): two whole fillers at
                    # each pair-group boundary. chunked=True (ACT-paced
                    # stream): filler/proj chunk lists drain one ~1us chunk
                    # per unit so no PE burst outruns the 1-exp lookahead.
                    fq_ = list(fillers)
                    work = []

                    def pump(n=1):
                        for _ in range(n):
                            if work:
                                work.pop(0)(psy)

                    ps_i = emit_scores(pss, units[0])
                    e_i = emit_exp(units[0], ps_i)
                    for i, u in enumerate(units):
                        half, fq, qc, kt = u
                        if half not in yn_cur:
                            yn_cur[half] = [
                                ynp.tile([128, 1024], f16, tag=f"yn{fc}", name=f"yn{fc}")
                                for fc in range(4)
                            ]
                        if i + 1 < len(units):
                            ps_n = emit_scores(pss, units[i + 1])
                            e_n = emit_exp(units[i + 1], ps_n)
                        if chunked:
                            # a group-start y waits its py slot (freed by the
                            # previous group's norm copy): queue extra PE work
                            # in front of it; otherwise drip every other unit
                            pump(2 if kt == 0 else (1 if i % 2 == 0 else 0))
                        emit_y(psy, u, e_i)
                        if kt == 8 * half + 4 * qc + 3:
                            emit_norm(u)
                            if chunked:
                                if fq_:
                                    work.extend(fq_.pop(0))
                                pump(1)
                            else:
                                for _ in range(2):
                                    if fq_:
                                        for ch in fq_.pop(0):
                                            ch(psy)
                        if (obp is not None and proj_q and i % 9 == 0
                                and len(work) < 4):
                            work.extend(proj_chunks(proj_q.pop(0), obp))
                        if i + 1 < len(units):
                            e_i = e_n
                    while fq_:
                        work.extend(fq_.pop(0))
                    if obp is not None:
                        # pair-interleave the tail projections with fc3 (the
                        # only part gated on the final norm) split out, so
                        # ~2.7us of norm-independent matmuls run first and
                        # the PE never idles into a HAM re-throttle
                        tails = []
                        while proj_q:
                            tails.append(proj_chunks(proj_q.pop(0), obp,
                                                     split=True))
                        for j in range(0, len(tails) - 1, 2):
                            a, b = tails[j], tails[j + 1]
                            work.extend([a[0], b[0], a[1], b[1],
                                         a[2], a[3], b[2], b[3]])
                        if len(tails) % 2:
                            work.extend(tails[-1])
                    # tail drain allocates from the SCORES pool slots (tag
                    # "s", same [128,1024] shape): those free the moment the
                    # last exp has read them, while psy's slot only frees
                    # after the final norm's DVE copies - draining from psy
                    # left the PE idle ~2.5us there, which also dropped the
                    # HAM clock to 1.2GHz for the whole projection tail
                    class _STag:
                        def tile(self, shape, dtype, tag=None, name=None):
                            return pss.tile(shape, dtype, tag="s", name=name)
                    stag = _STag()
                    while work:
                        work.pop(0)(stag)

                def make_units(half):
                    if half == 0:
                        return [(0, fq, qc, kt)
                                for fq in range(4)
                                for qc in range(2)
                                for kt in range(4 * qc + 4)]
                    # half 1: qc-outer so the v12-15 fillers (emitted at the
                    # first group boundaries) land before any kt>=12 unit
                    return [(1, fq, qc, kt)
                            for qc in range(2)
                            for fq in range(4)
                            for kt in range(8 + 4 * qc + 4)]

                units0 = make_units(0)
                units1 = make_units(1)

                # interleaved QKV work items for tokens 1024..2047.
                # q/k features ordered (0,4),(1,5),.. so half-1's fq-ordered
                # groups see their features early; v12-15 move to half-1's
                # stream (only kt>=12 units need them).
                fillers0 = []
                for f in (0, 4, 1, 5, 2, 6, 3, 7):
                    fillers0.append(qk_feature_chunks(f, x23, 1024))
                for tl in range(4):
                    fillers0.append(v_tile_chunks(tl, x23, 8 + tl, act_copy=True))
                fillers1 = [v_tile_chunks(tl, x23, 8 + tl) for tl in range(4, 8)]

                with (
                    tc.tile_pool(name="pss0", bufs=2, space="PSUM") as pss0,
                    tc.tile_pool(name="psy0", bufs=2, space="PSUM") as psy0,
                ):
                    run_stream(pss0, psy0, units0, fillers0, chunked=False)

                with (
                    tc.tile_pool(name="obp", bufs=2) as obp,
                    tc.tile_pool(name="pss1", bufs=2, space="PSUM") as pss1,
                    tc.tile_pool(name="psy1", bufs=2, space="PSUM") as psy1,
                ):
                    run_stream(pss1, psy1, units1, fillers1, obp=obp,
                               chunked=True)

    nc.compile()
    return nc


def _get_nc():
    if "nc" not in _CACHE:
        _CACHE["nc"] = _build_nc()
    return _CACHE["nc"]


def prepare_in_maps(x, W_attn, b_attn, W_proj, b_proj):
    import ml_dtypes
    x = np.asarray(x, dtype=np.float32)
    W_attn = np.asarray(W_attn, dtype=np.float32)
    b_attn = np.asarray(b_attn, dtype=np.float32)
    W_proj = np.asarray(W_proj, dtype=np.float32)

    mask = np.zeros((128, 256), np.float32)
    mask[:, 0:128] = np.triu(np.ones((128, 128), np.float32), 1)
    mask[:, 128:256] = -1e30 * np.eye(128, dtype=np.float32)
    maskb = np.ascontiguousarray(mask.astype(ml_dtypes.bfloat16))
    ones = np.ones((1, 128), np.float16)
    xts = [np.ascontiguousarray(x[b].T.astype(np.float16)) for b in range(4)]

    in_maps = []
    for c in range(8):
        b, hg = divmod(c, 2)
        s = hg * 512
        wqkv = np.ascontiguousarray(np.concatenate(
            [W_attn[:, s:s + 512],
             W_attn[:, 1024 + s:1024 + s + 512],
             W_attn[:, 2048 + s:2048 + s + 512]], axis=1).astype(np.float16))
        bqk = np.ascontiguousarray(
            np.concatenate([b_attn[s:s + 512], b_attn[1024 + s:1024 + s + 512]])
            .reshape(8, 128).T)
        bv = np.ascontiguousarray(
            b_attn[2048 + s:2048 + s + 512].reshape(1, 512).astype(np.float16))
        wproj = np.ascontiguousarray(
            W_proj[s:s + 512, :].astype(np.float16))
        in_maps.append({"xt": xts[b], "wqkv": wqkv, "bqk": bqk, "bv": bv,
                        "wproj": wproj, "ones": ones, "maskb": maskb})
    return in_maps


def kernel(x, W_attn, b_attn, W_proj, b_proj):
    from concourse.bass_utils import run_bass_kernel_spmd

    b_proj = np.asarray(b_proj, dtype=np.float32)
    nc = _get_nc()
    in_maps = prepare_in_maps(x, W_attn, b_attn, W_proj, b_proj)

    res = run_bass_kernel_spmd(nc, in_maps, core_ids=list(range(8)))
    y = np.empty((4, T, C), np.float32)
    for b in range(4):
        y[b] = (res.results[2 * b]["out"].astype(np.float32)
                + res.results[2 * b + 1]["out"].astype(np.float32) + b_proj)
    return y


# revision 45
# speedup vs baseline: 1.0019x; 1.0019x over previous
"""Causal self-attention (B=4, T=2048, C=1024, 16 heads) on 8 trn2 NeuronCores.

Sharding: core c handles batch b = c//2 and head-group hg = c%2 (8 of 16 heads).
Each core computes QKV projection for its heads, causal attention, and a partial
output projection (row-sharded W_proj); the host sums the two partials per batch
and adds b_proj.

Device layout notes:
 - x is fed pre-transposed ([C, T]) so the contraction dim C lands on SBUF
   partitions with no on-device transpose.
 - Scores are computed transposed (S^T[k, q]) so softmax's reduction over k can
   be done by the PE via a ones-column appended to V (row k of S^T is a
   partition; summing over partitions is a matmul).
 - Softmax skips the max-subtraction: scores/8 are ~N(0,1) here, exp is safe in
   fp32 and the result is mathematically identical.
 - All matmul operands are fp16 (fp32 PSUM accumulate): same PE stream rate as
   fp32, but FWL (fast weight load) halves LDWEIGHTS time, and SBUF/DMA
   traffic halves. fp16's 11-bit mantissa keeps end-to-end rel err ~3e-3.

Performance structure (v14, ~298us vs the 405us v4 baseline), built around
three engine limits measured in traces: PE matmul streaming (~213ns per
N=512), per-matmul LDWEIGHTS serialization (fp16 FWL halves it), and the ACT
engine's exp cost ((N+352)/1.2 ns per instruction):
 - Scores matmuls have K=64 (head dim): the two heads of a feature-pair (fq)
   live on partitions 0-63 / 64-127, so their score MMs target disjoint PE
   row-groups (tile_position auto-derived from base_partition) and run
   CONCURRENTLY when issued back-to-back - halving scores PE time.
 - Attention is organized in pair-units (half, fq, qc, kt) where qc is a
   512-token q chunk: one PSUM tile [128, 1024] holds both heads' scores
   (A: cols 0-511, B: 512-1023), so ONE exp instruction covers two heads
   (fewer ACT fixed overheads). Unwritten diag-trim columns are exp'd as
   garbage but never streamed into the y matmuls.
 - y accumulates per pair-group into a [65, 1024] PSUM region (rows 0-63 y,
   row 64 rowsum via the V ones-column; A cols 0-511, B 512-1023).
 - Normalization per pair-group: rowsum row AND unnormalized y rows copied
   to SBUF first (two plain-shape DVE copies: py's PSUM slot frees after
   ~2.3us instead of the full ~5.5us chain; a single [65,1024] copy produced
   NaN columns on HW - keep the two-copy form), then reciprocal_approx_fast
   (DVE; its bit-trick seed misreads PSUM), partition_broadcast (GpSimd;
   input must be a partition-0 tile, GpSimd cannot read PSUM), and two DVE
   multiplies -> yn (fp16).
 - Phase 1a (QKV for tokens 0..1023): chunk-outer loop over 8 PSUM banks so
   the first matmul starts right after the first w/x chunk DMA lands; range 1
   walks banks in reverse (they free in reverse order); v copies run on the
   otherwise-idle ACT engine (which, unlike GpSimd, can read PSUM).
 - Half-0's attention stream is PE-bound: QKV for tokens 1024..2047 is
   interleaved as whole fillers at pair-group boundaries. Half-1's stream is
   ACT-paced: exp(i+1) is issued before y(i) so ACT always has one queued
   exp, and filler/projection work drips in <=~1us chunks (one per unit, two
   ahead of a group-start y) so no PE burst outruns that 1-exp lookahead.
 - Output projection: half-0's tiles and half-1's qc0 tiles drip into half
   1's stream (chunked, through the psy pool); half-1's qc1 tiles are the
   tail. Output partials are fp16, summed in fp32 on the host. Output DMA
   overlaps compute. The dense PE queue also keeps the HAM clock at 2.4GHz.

HW exec time varies run-to-run (~0.5% warm; occasionally ~1.2x when the chip
enters the P0 power-state downclock) - compare kernels by best-of-3.
"""
import numpy as np

T = 2048          # tokens per batch element
C = 1024          # embed dim
H = 8             # heads per core
D = 64            # head dim
CC = 8            # contraction chunks (C / 128)

_CACHE = {}


def _build_nc():
    from concourse import bacc
    import concourse.mybir as mybir
    import concourse.tile as tile

    f32 = mybir.dt.float32
    f16 = mybir.dt.float16
    bf16 = mybir.dt.bfloat16
    EXP = mybir.ActivationFunctionType.Exp

    nc = bacc.Bacc("TRN2", num_devices=8, debug=False)

    xt_d = nc.dram_tensor("xt", [C, T], f16, kind="ExternalInput")
    wqkv_d = nc.dram_tensor("wqkv", [C, 1536], f16, kind="ExternalInput")
    bqk_d = nc.dram_tensor("bqk", [128, 8], f32, kind="ExternalInput")
    bv_d = nc.dram_tensor("bv", [1, 512], f16, kind="ExternalInput")
    wproj_d = nc.dram_tensor("wproj", [512, C], f16, kind="ExternalInput")
    ones_d = nc.dram_tensor("ones", [1, 128], f16, kind="ExternalInput")
    maskb_d = nc.dram_tensor("maskb", [128, 256], bf16, kind="ExternalInput")
    out_d = nc.dram_tensor("out", [T, C], f16, kind="ExternalOutput")

    with tile.TileContext(nc) as tc:
      with tc.tile_pool(name="persist", bufs=1) as pp:
        # persistent SBUF: qk^T [1024 feats, T] f16, v [T, 8*(64+1)] f16
        qk_sb = [pp.tile([128, T], f16, tag=f"qk{f}", name=f"qk{f}") for f in range(8)]
        v_sb = [pp.tile([128, H * 65], f16, tag=f"v{t}", name=f"v{t}") for t in range(16)]
        wp_sb = [pp.tile([128, C], f16, tag=f"wp{i}", name=f"wp{i}") for i in range(4)]
        maskb_sb = pp.tile([128, 256], bf16, tag="maskb")
        ones_sb = pp.tile([1, 128], f16, tag="ones")
        bqk_sb = pp.tile([128, 8], f32, tag="bqk")
        bv_sb = pp.tile([1, 512], f16, tag="bv")

        def persist_dmas():
            # issued AFTER the first w/x chunk DMAs: nothing here is needed
            # until attention / projection, so keep it off the critical path
            nc.sync.dma_start(maskb_sb[:], maskb_d[:])
            nc.sync.dma_start(bqk_sb[:], bqk_d[:])
            for i in range(4):
                nc.sync.dma_start(wp_sb[i][:], wproj_d[i * 128:(i + 1) * 128, :])

        for t in range(16):
            # ones column at position 64 of each head's 65-wide V block
            nc.gpsimd.memset(
                v_sb[t][:].rearrange("p (h e) -> p h e", e=65)[:, :, 64:65], 1.0
            )

        # Filler work (QKV for tokens 1024..2047, and the output projection)
        # is emitted as CHUNK LISTS: closures each costing <=~1us of PE time,
        # dripped one-per-attention-unit so no single PE burst outruns the
        # 1-exp ACT lookahead (which would stall the ACT-paced pipeline).
        def qk_feature_chunks(f, xs2, dst):
            # q/k features f*128..f*128+128 for tokens dst..dst+1024
            st = {}

            def mm_chunk(h, cr):
                def c_(pool):
                    if h == 0 and cr == 0:
                        st["pq"] = pool.tile([128, 1024], f32, tag="py", name="pq")
                    for c in range(cr, cr + 4):
                        nc.tensor.matmul(
                            st["pq"][:, h * 512:(h + 1) * 512],
                            w_sb[c][:, f * 128:(f + 1) * 128],
                            xs2[h][c][:],
                            start=(c == 0), stop=(c == CC - 1),
                        )
                return c_

            def c_add(pool):
                nc.vector.tensor_scalar_add(
                    qk_sb[f][:, dst:dst + 1024], st["pq"][:], bqk_sb[:, f:f + 1]
                )

            return [mm_chunk(0, 0), mm_chunk(0, 4),
                    mm_chunk(1, 0), mm_chunk(1, 4), c_add]

        def v_tile_chunks(tl, xs2, tg, act_copy=False):
            # v for 128 tokens (tl-th 128-block of xs2) -> v_sb[tg]
            xs = xs2[tl // 4]
            t0 = (tl % 4) * 128
            st = {}

            def c0(pool):
                st["pv"] = pool.tile([128, 1024], f32, tag="py", name="pv")
                for c in range(4):
                    nc.tensor.matmul(
                        st["pv"][:, 0:512], xs[c][:, t0:t0 + 128],
                        w_sb[c][:, 1024:1536],
                        start=(c == 0), stop=False,
                    )

            def c1(pool):
                for c in range(4, CC):
                    nc.tensor.matmul(
                        st["pv"][:, 0:512], xs[c][:, t0:t0 + 128],
                        w_sb[c][:, 1024:1536],
                        start=False, stop=False,
                    )
                nc.tensor.matmul(st["pv"][:, 0:512], ones_sb[:], bv_sb[:],
                                 start=False, stop=True)

            def c2(pool):
                # in stream 0, DVE runs hot (norm chains + bias adds) while
                # ACT has slack and can read PSUM -> copy on ACT there; in
                # the ACT-bound stream 1, keep the copy on DVE
                if act_copy:
                    nc.scalar.copy(
                        v_sb[tg][:].rearrange("p (h e) -> p h e", e=65)[:, :, 0:64],
                        st["pv"][:, 0:512].rearrange("p (h e) -> p h e", e=64),
                    )
                else:
                    nc.vector.tensor_copy(
                        v_sb[tg][:].rearrange("p (h e) -> p h e", e=65)[:, :, 0:64],
                        st["pv"][:, 0:512].rearrange("p (h e) -> p h e", e=64),
                    )

            return [c0, c1, c2]

        # ---------------- Phase 1a: QKV for tokens 0..1023 (ranges 0,1) -----
        with (
            tc.tile_pool(name="ynp", bufs=2) as ynp,
            tc.tile_pool(name="epool", bufs=4) as ep,
            tc.tile_pool(name="rpool", bufs=2) as rp,
        ):
          yn_cur = {}
          py_cur = {}
          proj_q = []
          with (
            tc.tile_pool(name="w", bufs=1) as pw,
            tc.tile_pool(name="xa", bufs=2) as pxa,
          ):
            w_sb = [pw.tile([128, 1536], f16, tag=f"w{c}", name=f"w{c}") for c in range(CC)]
            with (
                tc.tile_pool(name="psA", bufs=1, space="PSUM") as psA,
            ):
                x_r = {}
                for c in range(CC):
                    # pair chunk DMAs so the first matmul group starts early
                    nc.sync.dma_start(w_sb[c][:], wqkv_d[c * 128:(c + 1) * 128, :])
                    t_ = pxa.tile([128, 512], f16, tag=f"x{c}", name=f"x{c}")
                    nc.sync.dma_start(t_[:], xt_d[c * 128:(c + 1) * 128, 0:512])
                    x_r.setdefault(0, []).append(t_)
                    if c == 0:
                        # tiny, needed a few us in by the v-tile bias matmul
                        nc.sync.dma_start(ones_sb[:], ones_d[:])
                        nc.sync.dma_start(bv_sb[:], bv_d[:])
                    if c == CC - 1:
                        persist_dmas()
                for r in (0, 1):
                    if r == 1:
                        x_r[1] = []
                        for c in range(CC):
                            t_ = pxa.tile([128, 512], f16, tag=f"x{c}", name=f"x{c}")
                            nc.sync.dma_start(
                                t_[:], xt_d[c * 128:(c + 1) * 128, 512:1024])
                            x_r[1].append(t_)
                    # chunk-outer over 8 psum banks: chunk c usable on
                    # arrival. Range 1 walks features in reverse so it starts
                    # on the banks range 0 freed first (qk adds finish before
                    # the v copies on banks 0-3).
                    forder = list(range(8)) if r == 0 else list(range(7, -1, -1))
                    tlorder = list(range(4)) if r == 0 else list(range(3, -1, -1))
                    pq8 = {f: psA.tile([128, 512], f32, tag=f"b{f}", name=f"b{f}")
                           for f in forder}
                    for c in range(CC):
                        for f in forder:
                            nc.tensor.matmul(
                                pq8[f][:], w_sb[c][:, f * 128:(f + 1) * 128],
                                x_r[r][c][:],
                                start=(c == 0), stop=(c == CC - 1),
                            )
                    for f in forder:
                        nc.vector.tensor_scalar_add(
                            qk_sb[f][:, r * 512:(r + 1) * 512], pq8[f][:],
                            bqk_sb[:, f:f + 1],
                        )
                    for tl in tlorder:
                        tg = r * 4 + tl
                        pv = psA.tile([128, 512], f32, tag=f"b{tl}", name=f"pv{tl}")
                        for c in range(CC):
                            nc.tensor.matmul(
                                pv[:], x_r[r][c][:, tl * 128:(tl + 1) * 128],
                                w_sb[c][:, 1024:1536],
                                start=(c == 0), stop=False,
                            )
                        nc.tensor.matmul(pv[:], ones_sb[:], bv_sb[:],
                                         start=False, stop=True)
                        # ACT is idle in phase 1a and can read PSUM: v copies
                        # go there so the DVE only carries the qk bias-adds
                        nc.scalar.copy(
                            v_sb[tg][:].rearrange("p (h e) -> p h e", e=65)[:, :, 0:64],
                            pv[:].rearrange("p (h e) -> p h e", e=64),
                        )

            # x for tokens 1024..2047 (ranges 2,3), used by the interleaved
            # QKV: two more generations of the xa pool's chunk tiles
            x23 = []
            for h, lo in enumerate((1024, 1536)):
                xs = []
                for c in range(CC):
                    t_ = pxa.tile([128, 512], f16, tag=f"x{c}", name=f"x{c}")
                    nc.sync.dma_start(t_[:], xt_d[c * 128:(c + 1) * 128, lo:lo + 512])
                    xs.append(t_)
                x23.append(xs)

            # ---------------- Phase 2: attention ----------------------------
            # Pair-units (half, fq, qc, kt): heads A=2fq (partitions 0-63)
            # and B=2fq+1 (64-127) computed together; their score MMs hit
            # disjoint PE row-groups and run concurrently.
            if True:
                def emit_scores(pss, u):
                    half, fq, qc, kt = u
                    qg = 1024 * half + 512 * qc
                    a = max(0, kt * 128 - qg)
                    diag = kt >= 8 * half + 4 * qc
                    kcol = slice(kt * 128, (kt + 1) * 128)
                    qcol = slice(qg + a, qg + 512)
                    ps = pss.tile([128, 1024], f32, tag="s", name="ps_s")
                    nc.tensor.matmul(
                        ps[0:128, a:512],
                        qk_sb[4 + fq][0:64, kcol], qk_sb[fq][0:64, qcol],
                        start=True, stop=not diag,
                    )
                    nc.tensor.matmul(
                        ps[0:128, 512 + a:1024],
                        qk_sb[4 + fq][64:128, kcol], qk_sb[fq][64:128, qcol],
                        start=True, stop=not diag,
                    )
                    if diag:
                        # += -1e30 * upper_strict on each head's diag block
                        nc.tensor.matmul(
                            ps[:, a:a + 128],
                            maskb_sb[:, 0:128], maskb_sb[:, 128:256],
                            start=False, stop=True,
                        )
                        nc.tensor.matmul(
                            ps[:, 512 + a:512 + a + 128],
                            maskb_sb[:, 0:128], maskb_sb[:, 128:256],
                            start=False, stop=True,
                        )
                    return ps

                def emit_exp(u, ps):
                    half, fq, qc, kt = u
                    qg = 1024 * half + 512 * qc
                    a = max(0, kt * 128 - qg)
                    # one instruction covers both heads' valid regions
                    # ([a,512) and [512+a,1024)); cols [512-a,512) of e are
                    # exp(stale PSUM) and are never streamed into y.
                    e = ep.tile([128, 1024], f16, tag="e", name="e_t")
                    nc.scalar.activation(
                        e[:, 0:1024 - a], ps[:, a:1024], EXP, scale=0.125,
                    )
                    return e

                def emit_y(psy, u, e):
                    half, fq, qc, kt = u
                    qg = 1024 * half + 512 * qc
                    a = max(0, kt * 128 - qg)
                    last = 8 * half + 4 * qc + 3
                    if kt == 0:
                        py_cur[(half, fq, qc)] = psy.tile(
                            [128, 1024], f32, tag="py", name="py_t")
                    py = py_cur[(half, fq, qc)]
                    nc.tensor.matmul(
                        py[0:65, a:512],
                        v_sb[kt][:, (2 * fq) * 65:(2 * fq + 1) * 65],
                        e[:, 0:512 - a],
                        start=(kt == 0), stop=(kt == last),
                    )
                    nc.tensor.matmul(
                        py[0:65, 512 + a:1024],
                        v_sb[kt][:, (2 * fq + 1) * 65:(2 * fq + 2) * 65],
                        e[:, 512:1024 - a],
                        start=(kt == 0), stop=(kt == last),
                    )

                def emit_norm(u):
                    half, fq, qc, _ = u
                    py = py_cur.pop((half, fq, qc))
                    # ONE copy of py (y rows + rowsum row) to SBUF frees the
                    # PSUM slot after a single ~1.3us DVE op; the broadcast /
                    # reciprocal / normalize run lazily from the SBUF copy
                    # (the custom-DVE recip's bit-trick seed misreads PSUM,
                    # and GpSimd cannot read PSUM at all).
                    # stage rowsum AND unnormalized y to SBUF up front: py's
                    # PSUM slot frees after these two copies (~2.3us) instead
                    # of after the full recip/broadcast/mul chain (~5.5us),
                    # so the next group's y never waits on this group's norm
                    rs = rp.tile([1, 1024], f32, tag="rs", name="rs_t")
                    nc.vector.tensor_copy(rs[:], py[64:65, 0:1024])
                    ya = rp.tile([64, 1024], f32, tag="ya", name="ya_t")
                    nc.vector.tensor_copy(ya[:], py[0:64, 0:1024])
                    r = rp.tile([1, 1024], f32, tag="r", name="r_t")
                    nc.vector.reciprocal_approx_fast(r[:], rs[:])
                    rb = rp.tile([64, 1024], f32, tag="rb", name="rb_t")
                    nc.gpsimd.partition_broadcast(rb[:], r[:])
                    qcc = slice(qc * 512, qc * 512 + 512)
                    nc.vector.tensor_mul(
                        yn_cur[half][fq][0:64, qcc], ya[:, 0:512], rb[:, 0:512],
                    )
                    nc.vector.tensor_mul(
                        yn_cur[half][fq][64:128, qcc], ya[:, 512:1024],
                        rb[:, 512:1024],
                    )
                    if half == 1 and fq == 3:
                        # qc-outer ordering: all 4 fq done for this qc ->
                        # the qc's token blocks can be projected already
                        for tt in range(qc * 4, qc * 4 + 4):
                            proj_q.append((half, tt))
                    elif half == 0 and fq == 3 and qc == 1:
                        for tt in range(8):
                            proj_q.append((half, tt))

                def proj_chunks(item, obp, split=False):
                    half, tt = item
                    st = {}

                    def mm_chunk(fcs):
                        def c_(pool):
                            if fcs[0] == 0:
                                st["po"] = pool.tile(
                                    [128, 1024], f32, tag="py", name="po_t")
                            for fc in fcs:
                                for n in range(2):
                                    nc.tensor.matmul(
                                        st["po"][:, n * 512:(n + 1) * 512],
                                        yn_cur[half][fc][:, tt * 128:(tt + 1) * 128],
                                        wp_sb[fc][:, n * 512:(n + 1) * 512],
                                        start=(fc == 0), stop=(fc == 3),
                                    )
                        return c_

                    def c_out(pool):
                        ob = obp.tile([128, C], f16, tag="ob")
                        nc.vector.tensor_copy(ob[:], st["po"][:])
                        nc.sync.dma_start(
                            out_d[half * 1024 + tt * 128:
                                  half * 1024 + (tt + 1) * 128, :],
                            ob[:],
                        )

                    if split:
                        # tail form: fc3 (which waits the final norm) in its
                        # own chunk so fc0-2 work can bridge the norm latency
                        return [mm_chunk((0, 1)), mm_chunk((2,)),
                                mm_chunk((3,)), c_out]
                    return [mm_chunk((0, 1)), mm_chunk((2, 3)), c_out]

                def run_stream(pss, psy, units, fillers, obp=None,
                               chunked=False):
                    # scores(i+1) AND exp(i+1) are traced before y(i): the
                    # queued exp keeps ACT fed through PE bursts.
                    # chunked=False (PE-bound stream): two whole fillers at
                    # each pair-group boundary. chunked=True (ACT-paced
                    # stream): filler/proj chunk lists drain one ~1us chunk
                    # per unit so no PE burst outruns the 1-exp lookahead.
                    fq_ = list(fillers)
                    work = []

                    def pump(n=1):
                        for _ in range(n):
                            if work:
                                work.pop(0)(psy)

                    ps_i = emit_scores(pss, units[0])
                    e_i = emit_exp(units[0], ps_i)
                    for i, u in enumerate(units):
                        half, fq, qc, kt = u
                        if half not in yn_cur:
                            yn_cur[half] = [
                                ynp.tile([128, 1024], f16, tag=f"yn{fc}", name=f"yn{fc}")
                                for fc in range(4)
                            ]
                        if i + 1 < len(units):
                            ps_n = emit_scores(pss, units[i + 1])
                            e_n = emit_exp(units[i + 1], ps_n)
                        if chunked:
                            # a group-start y waits its py slot (freed by the
                            # previous group's norm copy): queue extra PE work
                            # in front of it; otherwise drip every other unit
                            pump(2 if kt == 0 else (1 if i % 2 == 0 else 0))
                        emit_y(psy, u, e_i)
                        if kt == 8 * half + 4 * qc + 3:
                            emit_norm(u)
                            if chunked:
                                if fq_:
                                    work.extend(fq_.pop(0))
                                pump(1)
                            else:
                                for _ in range(2):
                                    if fq_:
                                        for ch in fq_.pop(0):
                                            ch(psy)
                        if (obp is not None and proj_q and i % 9 == 0
                                and len(work) < 4):
                            work.extend(proj_chunks(proj_q.pop(0), obp))
                        if i + 1 < len(units):
                            e_i = e_n
                    while fq_:
                        work.extend(fq_.pop(0))
                    if obp is not None:
                        # pair-interleave the tail projections with fc3 (the
                        # only part gated on the final norm) split out, so
                        # ~2.7us of norm-independent matmuls run first and
                        # the PE never idles into a HAM re-throttle
                        tails = []
                        while proj_q:
                            tails.append(proj_chunks(proj_q.pop(0), obp,
                                                     split=True))
                        for j in range(0, len(tails) - 1, 2):
                            a, b = tails[j], tails[j + 1]
                            work.extend([a[0], b[0], a[1], b[1],
                                         a[2], a[3], b[2], b[3]])
                        if len(tails) % 2:
                            work.extend(tails[-1])
                    # tail drain allocates from the SCORES pool slots (tag
                    # "s", same [128,1024] shape): those free the moment the
                    # last exp has read them, while psy's slot only frees
                    # after the final norm's DVE copies - draining from psy
                    # left the PE idle ~2.5us there, which also dropped the
                    # HAM clock to 1.2GHz for the whole projection tail
                    class _STag:
                        def tile(self, shape, dtype, tag=None, name=None):
                            return pss.tile(shape, dtype, tag="s", name=name)
                    stag = _STag()
                    while work:
                        work.pop(0)(stag)

                def make_units(half):
                    if half == 0:
                        return [(0, fq, qc, kt)
                                for fq in range(4)
                                for qc in range(2)
                                for kt in range(4 * qc + 4)]
                    # half 1: qc-outer so the v12-15 fillers (emitted at the
                    # first group boundaries) land before any kt>=12 unit
                    return [(1, fq, qc, kt)
                            for qc in range(2)
                            for fq in range(4)
                            for kt in range(8 + 4 * qc + 4)]

                units0 = make_units(0)
                units1 = make_units(1)

                # interleaved QKV work items for tokens 1024..2047.
                # q/k features ordered (0,4),(1,5),.. so half-1's fq-ordered
                # groups see their features early; v12-15 move to half-1's
                # stream (only kt>=12 units need them).
                fillers0 = []
                for f in (0, 4, 1, 5, 2, 6, 3, 7):
                    fillers0.append(qk_feature_chunks(f, x23, 1024))
                for tl in range(4):
                    fillers0.append(v_tile_chunks(tl, x23, 8 + tl, act_copy=True))
                fillers1 = [v_tile_chunks(tl, x23, 8 + tl) for tl in range(4, 8)]

                with (
                    tc.tile_pool(name="pss0", bufs=2, space="PSUM") as pss0,
                    tc.tile_pool(name="psy0", bufs=2, space="PSUM") as psy0,
                ):
                    run_stream(pss0, psy0, units0, fillers0, chunked=False)

                with (
                    tc.tile_pool(name="obp", bufs=2) as obp,
                    tc.tile_pool(name="pss1", bufs=2, space="PSUM") as pss1,
                    tc.tile_pool(name="psy1", bufs=2, space="PSUM") as psy1,
                ):
                    run_stream(pss1, psy1, units1, fillers1, obp=obp,
                               chunked=True)

    nc.compile()
    return nc


def _get_nc():
    if "nc" not in _CACHE:
        _CACHE["nc"] = _build_nc()
    return _CACHE["nc"]


def prepare_in_maps(x, W_attn, b_attn, W_proj, b_proj):
    import ml_dtypes
    x = np.asarray(x, dtype=np.float32)
    W_attn = np.asarray(W_attn, dtype=np.float32)
    b_attn = np.asarray(b_attn, dtype=np.float32)
    W_proj = np.asarray(W_proj, dtype=np.float32)

    mask = np.zeros((128, 256), np.float32)
    mask[:, 0:128] = np.triu(np.ones((128, 128), np.float32), 1)
    mask[:, 128:256] = -1e30 * np.eye(128, dtype=np.float32)
    maskb = np.ascontiguousarray(mask.astype(ml_dtypes.bfloat16))
    ones = np.ones((1, 128), np.float16)
    xts = [np.ascontiguousarray(x[b].T.astype(np.float16)) for b in range(4)]

    in_maps = []
    for c in range(8):
        b, hg = divmod(c, 2)
        s = hg * 512
        wqkv = np.ascontiguousarray(np.concatenate(
            [W_attn[:, s:s + 512],
             W_attn[:, 1024 + s:1024 + s + 512],
             W_attn[:, 2048 + s:2048 + s + 512]], axis=1).astype(np.float16))
        bqk = np.ascontiguousarray(
            np.concatenate([b_attn[s:s + 512], b_attn[1024 + s:1024 + s + 512]])
            .reshape(8, 128).T)
        bv = np.ascontiguousarray(
            b_attn[2048 + s:2048 + s + 512].reshape(1, 512).astype(np.float16))
        wproj = np.ascontiguousarray(
            W_proj[s:s + 512, :].astype(np.float16))
        in_maps.append({"xt": xts[b], "wqkv": wqkv, "bqk": bqk, "bv": bv,
                        "wproj": wproj, "ones": ones, "maskb": maskb})
    return in_maps


def kernel(x, W_attn, b_attn, W_proj, b_proj):
    from concourse.bass_utils import run_bass_kernel_spmd

    b_proj = np.asarray(b_proj, dtype=np.float32)
    nc = _get_nc()
    in_maps = prepare_in_maps(x, W_attn, b_attn, W_proj, b_proj)

    res = run_bass_kernel_spmd(nc, in_maps, core_ids=list(range(8)))
    y = np.empty((4, T, C), np.float32)
    for b in range(4):
        y[b] = (res.results[2 * b]["out"].astype(np.float32)
                + res.results[2 * b + 1]["out"].astype(np.float32) + b_proj)
    return y


# revision 47
# speedup vs baseline: 1.0094x; 1.0075x over previous
"""Causal self-attention (B=4, T=2048, C=1024, 16 heads) on 8 trn2 NeuronCores.

Sharding: core c handles batch b = c//2 and head-group hg = c%2 (8 of 16 heads).
Each core computes QKV projection for its heads, causal attention, and a partial
output projection (row-sharded W_proj); the host sums the two partials per batch
and adds b_proj.

Device layout notes:
 - x is fed pre-transposed ([C, T]) so the contraction dim C lands on SBUF
   partitions with no on-device transpose.
 - Scores are computed transposed (S^T[k, q]) so softmax's reduction over k can
   be done by the PE via a ones-column appended to V (row k of S^T is a
   partition; summing over partitions is a matmul).
 - Softmax skips the max-subtraction: scores/8 are ~N(0,1) here, exp is safe in
   fp32 and the result is mathematically identical.
 - All matmul operands are fp16 (fp32 PSUM accumulate): same PE stream rate as
   fp32, but FWL (fast weight load) halves LDWEIGHTS time, and SBUF/DMA
   traffic halves. fp16's 11-bit mantissa keeps end-to-end rel err ~3e-3.

Performance structure (v14, ~298us vs the 405us v4 baseline), built around
three engine limits measured in traces: PE matmul streaming (~213ns per
N=512), per-matmul LDWEIGHTS serialization (fp16 FWL halves it), and the ACT
engine's exp cost ((N+352)/1.2 ns per instruction):
 - Scores matmuls have K=64 (head dim): the two heads of a feature-pair (fq)
   live on partitions 0-63 / 64-127, so their score MMs target disjoint PE
   row-groups (tile_position auto-derived from base_partition) and run
   CONCURRENTLY when issued back-to-back - halving scores PE time.
 - Attention is organized in pair-units (half, fq, qc, kt) where qc is a
   512-token q chunk: one PSUM tile [128, 1024] holds both heads' scores
   (A: cols 0-511, B: 512-1023), so ONE exp instruction covers two heads
   (fewer ACT fixed overheads). Unwritten diag-trim columns are exp'd as
   garbage but never streamed into the y matmuls.
 - y accumulates per pair-group into a [65, 1024] PSUM region (rows 0-63 y,
   row 64 rowsum via the V ones-column; A cols 0-511, B 512-1023).
 - Normalization per pair-group: rowsum row AND unnormalized y rows copied
   to SBUF first (two plain-shape DVE copies: py's PSUM slot frees after
   ~2.3us instead of the full ~5.5us chain; a single [65,1024] copy produced
   NaN columns on HW - keep the two-copy form), then reciprocal_approx_fast
   (DVE; its bit-trick seed misreads PSUM), partition_broadcast (GpSimd;
   input must be a partition-0 tile, GpSimd cannot read PSUM), and two DVE
   multiplies -> yn (fp16).
 - Phase 1a (QKV for tokens 0..1023): chunk-outer loop over 8 PSUM banks so
   the first matmul starts right after the first w/x chunk DMA lands; range 1
   walks banks in reverse (they free in reverse order); v copies run on the
   otherwise-idle ACT engine (which, unlike GpSimd, can read PSUM).
 - Half-0's attention stream is PE-bound: QKV for tokens 1024..2047 is
   interleaved as whole fillers at pair-group boundaries. Half-1's stream is
   ACT-paced: exp(i+1) is issued before y(i) so ACT always has one queued
   exp, and filler/projection work drips in <=~1us chunks (one per unit, two
   ahead of a group-start y) so no PE burst outruns that 1-exp lookahead.
 - Output projection: half-0's tiles and half-1's qc0 tiles drip into half
   1's stream (chunked, through the psy pool); half-1's qc1 tiles are the
   tail. Output partials are fp16, summed in fp32 on the host. Output DMA
   overlaps compute. The dense PE queue also keeps the HAM clock at 2.4GHz.

HW exec time varies run-to-run (~0.5% warm; occasionally ~1.2x when the chip
enters the P0 power-state downclock) - compare kernels by best-of-3.
"""
import numpy as np

T = 2048          # tokens per batch element
C = 1024          # embed dim
H = 8             # heads per core
D = 64            # head dim
CC = 8            # contraction chunks (C / 128)

_CACHE = {}


def _build_nc():
    from concourse import bacc
    import concourse.mybir as mybir
    import concourse.tile as tile

    f32 = mybir.dt.float32
    f16 = mybir.dt.float16
    bf16 = mybir.dt.bfloat16
    EXP = mybir.ActivationFunctionType.Exp

    nc = bacc.Bacc("TRN2", num_devices=8, debug=False)

    xt_d = nc.dram_tensor("xt", [C, T], f16, kind="ExternalInput")
    wqkv_d = nc.dram_tensor("wqkv", [C, 1536], f16, kind="ExternalInput")
    bqk_d = nc.dram_tensor("bqk", [128, 8], f32, kind="ExternalInput")
    bv_d = nc.dram_tensor("bv", [1, 512], f16, kind="ExternalInput")
    wproj_d = nc.dram_tensor("wproj", [512, C], f16, kind="ExternalInput")
    ones_d = nc.dram_tensor("ones", [1, 128], f16, kind="ExternalInput")
    maskb_d = nc.dram_tensor("maskb", [128, 256], bf16, kind="ExternalInput")
    out_d = nc.dram_tensor("out", [T, C], f16, kind="ExternalOutput")

    with tile.TileContext(nc) as tc:
      with tc.tile_pool(name="persist", bufs=1) as pp:
        # persistent SBUF: qk^T [1024 feats, T] f16, v [T, 8*(64+1)] f16
        qk_sb = [pp.tile([128, T], f16, tag=f"qk{f}", name=f"qk{f}") for f in range(8)]
        v_sb = [pp.tile([128, H * 65], f16, tag=f"v{t}", name=f"v{t}") for t in range(16)]
        wp_sb = [pp.tile([128, C], f16, tag=f"wp{i}", name=f"wp{i}") for i in range(4)]
        maskb_sb = pp.tile([128, 256], bf16, tag="maskb")
        ones_sb = pp.tile([1, 128], f16, tag="ones")
        bqk_sb = pp.tile([128, 8], f32, tag="bqk")
        bv_sb = pp.tile([1, 512], f16, tag="bv")

        def persist_dmas():
            # issued AFTER the first w/x chunk DMAs: nothing here is needed
            # until attention / projection, so keep it off the critical path
            nc.sync.dma_start(maskb_sb[:], maskb_d[:])
            nc.sync.dma_start(bqk_sb[:], bqk_d[:])
            for i in range(4):
                nc.sync.dma_start(wp_sb[i][:], wproj_d[i * 128:(i + 1) * 128, :])

        for t in range(16):
            # ones column at position 64 of each head's 65-wide V block
            nc.gpsimd.memset(
                v_sb[t][:].rearrange("p (h e) -> p h e", e=65)[:, :, 64:65], 1.0
            )

        # Filler work (QKV for tokens 1024..2047, and the output projection)
        # is emitted as CHUNK LISTS: closures each costing <=~1us of PE time,
        # dripped one-per-attention-unit so no single PE burst outruns the
        # 1-exp ACT lookahead (which would stall the ACT-paced pipeline).
        def qk_feature_chunks(f, xs2, dst):
            # q/k features f*128..f*128+128 for tokens dst..dst+1024
            st = {}

            def mm_chunk(h, cr):
                def c_(pool):
                    if h == 0 and cr == 0:
                        st["pq"] = pool.tile([128, 1024], f32, tag="py", name="pq")
                    for c in range(cr, cr + 4):
                        nc.tensor.matmul(
                            st["pq"][:, h * 512:(h + 1) * 512],
                            w_sb[c][:, f * 128:(f + 1) * 128],
                            xs2[h][c][:],
                            start=(c == 0), stop=(c == CC - 1),
                        )
                return c_

            def c_add(pool):
                nc.vector.tensor_scalar_add(
                    qk_sb[f][:, dst:dst + 1024], st["pq"][:], bqk_sb[:, f:f + 1]
                )

            return [mm_chunk(0, 0), mm_chunk(0, 4),
                    mm_chunk(1, 0), mm_chunk(1, 4), c_add]

        def v_tile_chunks(tl, xs2, tg, act_copy=False):
            # v for 128 tokens (tl-th 128-block of xs2) -> v_sb[tg]
            xs = xs2[tl // 4]
            t0 = (tl % 4) * 128
            st = {}

            def c0(pool):
                st["pv"] = pool.tile([128, 1024], f32, tag="py", name="pv")
                for c in range(4):
                    nc.tensor.matmul(
                        st["pv"][:, 0:512], xs[c][:, t0:t0 + 128],
                        w_sb[c][:, 1024:1536],
                        start=(c == 0), stop=False,
                    )

            def c1(pool):
                for c in range(4, CC):
                    nc.tensor.matmul(
                        st["pv"][:, 0:512], xs[c][:, t0:t0 + 128],
                        w_sb[c][:, 1024:1536],
                        start=False, stop=False,
                    )
                nc.tensor.matmul(st["pv"][:, 0:512], ones_sb[:], bv_sb[:],
                                 start=False, stop=True)

            def c2(pool):
                # in stream 0, DVE runs hot (norm chains + bias adds) while
                # ACT has slack and can read PSUM -> copy on ACT there; in
                # the ACT-bound stream 1, keep the copy on DVE
                if act_copy:
                    nc.scalar.copy(
                        v_sb[tg][:].rearrange("p (h e) -> p h e", e=65)[:, :, 0:64],
                        st["pv"][:, 0:512].rearrange("p (h e) -> p h e", e=64),
                    )
                else:
                    nc.vector.tensor_copy(
                        v_sb[tg][:].rearrange("p (h e) -> p h e", e=65)[:, :, 0:64],
                        st["pv"][:, 0:512].rearrange("p (h e) -> p h e", e=64),
                    )

            return [c0, c1, c2]

        # ---------------- Phase 1a: QKV for tokens 0..1023 (ranges 0,1) -----
        with (
            tc.tile_pool(name="ynp", bufs=2) as ynp,
            tc.tile_pool(name="epool", bufs=4) as ep,
            tc.tile_pool(name="rpool", bufs=2) as rp,
        ):
          yn_cur = {}
          py_cur = {}
          proj_q = []
          with (
            tc.tile_pool(name="w", bufs=1) as pw,
            tc.tile_pool(name="xa", bufs=2) as pxa,
          ):
            w_sb = [pw.tile([128, 1536], f16, tag=f"w{c}", name=f"w{c}") for c in range(CC)]
            with (
                tc.tile_pool(name="psA", bufs=1, space="PSUM") as psA,
            ):
                x_r = {}
                for c in range(CC):
                    # pair chunk DMAs so the first matmul group starts early
                    nc.sync.dma_start(w_sb[c][:], wqkv_d[c * 128:(c + 1) * 128, :])
                    t_ = pxa.tile([128, 512], f16, tag=f"x{c}", name=f"x{c}")
                    nc.sync.dma_start(t_[:], xt_d[c * 128:(c + 1) * 128, 0:512])
                    x_r.setdefault(0, []).append(t_)
                    if c == 0:
                        # tiny, needed a few us in by the v-tile bias matmul
                        nc.sync.dma_start(ones_sb[:], ones_d[:])
                        nc.sync.dma_start(bv_sb[:], bv_d[:])
                    if c == CC - 1:
                        persist_dmas()
                for r in (0, 1):
                    if r == 1:
                        x_r[1] = []
                        for c in range(CC):
                            t_ = pxa.tile([128, 512], f16, tag=f"x{c}", name=f"x{c}")
                            nc.sync.dma_start(
                                t_[:], xt_d[c * 128:(c + 1) * 128, 512:1024])
                            x_r[1].append(t_)
                    # chunk-outer over 8 psum banks: chunk c usable on
                    # arrival. Range 1 walks features in reverse so it starts
                    # on the banks range 0 freed first (qk adds finish before
                    # the v copies on banks 0-3).
                    forder = list(range(8)) if r == 0 else list(range(7, -1, -1))
                    tlorder = list(range(4)) if r == 0 else list(range(3, -1, -1))
                    pq8 = {f: psA.tile([128, 512], f32, tag=f"b{f}", name=f"b{f}")
                           for f in forder}
                    for c in range(CC):
                        for f in forder:
                            nc.tensor.matmul(
                                pq8[f][:], w_sb[c][:, f * 128:(f + 1) * 128],
                                x_r[r][c][:],
                                start=(c == 0), stop=(c == CC - 1),
                            )
                    for f in forder:
                        nc.vector.tensor_scalar_add(
                            qk_sb[f][:, r * 512:(r + 1) * 512], pq8[f][:],
                            bqk_sb[:, f:f + 1],
                        )
                    for tl in tlorder:
                        tg = r * 4 + tl
                        pv = psA.tile([128, 512], f32, tag=f"b{tl}", name=f"pv{tl}")
                        for c in range(CC):
                            nc.tensor.matmul(
                                pv[:], x_r[r][c][:, tl * 128:(tl + 1) * 128],
                                w_sb[c][:, 1024:1536],
                                start=(c == 0), stop=False,
                            )
                        nc.tensor.matmul(pv[:], ones_sb[:], bv_sb[:],
                                         start=False, stop=True)
                        # ACT is idle in phase 1a and can read PSUM: v copies
                        # go there so the DVE only carries the qk bias-adds
                        nc.scalar.copy(
                            v_sb[tg][:].rearrange("p (h e) -> p h e", e=65)[:, :, 0:64],
                            pv[:].rearrange("p (h e) -> p h e", e=64),
                        )

            # x for tokens 1024..2047 (ranges 2,3), used by the interleaved
            # QKV: two more generations of the xa pool's chunk tiles
            x23 = []
            for h, lo in enumerate((1024, 1536)):
                xs = []
                for c in range(CC):
                    t_ = pxa.tile([128, 512], f16, tag=f"x{c}", name=f"x{c}")
                    nc.sync.dma_start(t_[:], xt_d[c * 128:(c + 1) * 128, lo:lo + 512])
                    xs.append(t_)
                x23.append(xs)

            # ---------------- Phase 2: attention ----------------------------
            # Pair-units (half, fq, qc, kt): heads A=2fq (partitions 0-63)
            # and B=2fq+1 (64-127) computed together; their score MMs hit
            # disjoint PE row-groups and run concurrently.
            if True:
                def emit_scores(pss, u):
                    half, fq, qc, kt = u
                    qg = 1024 * half + 512 * qc
                    a = max(0, kt * 128 - qg)
                    diag = kt >= 8 * half + 4 * qc
                    kcol = slice(kt * 128, (kt + 1) * 128)
                    qcol = slice(qg + a, qg + 512)
                    ps = pss.tile([128, 1024], f32, tag="s", name="ps_s")
                    nc.tensor.matmul(
                        ps[0:128, a:512],
                        qk_sb[4 + fq][0:64, kcol], qk_sb[fq][0:64, qcol],
                        start=True, stop=not diag,
                    )
                    nc.tensor.matmul(
                        ps[0:128, 512 + a:1024],
                        qk_sb[4 + fq][64:128, kcol], qk_sb[fq][64:128, qcol],
                        start=True, stop=not diag,
                    )
                    if diag:
                        # += -1e30 * upper_strict on each head's diag block
                        nc.tensor.matmul(
                            ps[:, a:a + 128],
                            maskb_sb[:, 0:128], maskb_sb[:, 128:256],
                            start=False, stop=True,
                        )
                        nc.tensor.matmul(
                            ps[:, 512 + a:512 + a + 128],
                            maskb_sb[:, 0:128], maskb_sb[:, 128:256],
                            start=False, stop=True,
                        )
                    return ps

                def emit_exp(u, ps):
                    half, fq, qc, kt = u
                    qg = 1024 * half + 512 * qc
                    a = max(0, kt * 128 - qg)
                    # one instruction covers both heads' valid regions
                    # ([a,512) and [512+a,1024)); cols [512-a,512) of e are
                    # exp(stale PSUM) and are never streamed into y.
                    e = ep.tile([128, 1024], f16, tag="e", name="e_t")
                    nc.scalar.activation(
                        e[:, 0:1024 - a], ps[:, a:1024], EXP, scale=0.125,
                    )
                    return e

                def emit_y(psy, u, e):
                    half, fq, qc, kt = u
                    qg = 1024 * half + 512 * qc
                    a = max(0, kt * 128 - qg)
                    last = 8 * half + 4 * qc + 3
                    if kt == 0:
                        py_cur[(half, fq, qc)] = psy.tile(
                            [128, 1024], f32, tag="py", name="py_t")
                    py = py_cur[(half, fq, qc)]
                    nc.tensor.matmul(
                        py[0:65, a:512],
                        v_sb[kt][:, (2 * fq) * 65:(2 * fq + 1) * 65],
                        e[:, 0:512 - a],
                        start=(kt == 0), stop=(kt == last),
                    )
                    nc.tensor.matmul(
                        py[0:65, 512 + a:1024],
                        v_sb[kt][:, (2 * fq + 1) * 65:(2 * fq + 2) * 65],
                        e[:, 512:1024 - a],
                        start=(kt == 0), stop=(kt == last),
                    )

                def emit_norm(u):
                    half, fq, qc, _ = u
                    py = py_cur.pop((half, fq, qc))
                    # ONE copy of py (y rows + rowsum row) to SBUF frees the
                    # PSUM slot after a single ~1.3us DVE op; the broadcast /
                    # reciprocal / normalize run lazily from the SBUF copy
                    # (the custom-DVE recip's bit-trick seed misreads PSUM,
                    # and GpSimd cannot read PSUM at all).
                    # stage rowsum AND unnormalized y to SBUF up front: py's
                    # PSUM slot frees after these two copies (~2.3us) instead
                    # of after the full recip/broadcast/mul chain (~5.5us),
                    # so the next group's y never waits on this group's norm
                    rs = rp.tile([1, 1024], f32, tag="rs", name="rs_t")
                    nc.vector.tensor_copy(rs[:], py[64:65, 0:1024])
                    ya = rp.tile([64, 1024], f32, tag="ya", name="ya_t")
                    nc.vector.tensor_copy(ya[:], py[0:64, 0:1024])
                    r = rp.tile([1, 1024], f32, tag="r", name="r_t")
                    nc.vector.reciprocal_approx_fast(r[:], rs[:])
                    rb = rp.tile([64, 1024], f32, tag="rb", name="rb_t")
                    nc.gpsimd.partition_broadcast(rb[:], r[:])
                    qcc = slice(qc * 512, qc * 512 + 512)
                    nc.vector.tensor_mul(
                        yn_cur[half][fq][0:64, qcc], ya[:, 0:512], rb[:, 0:512],
                    )
                    nc.vector.tensor_mul(
                        yn_cur[half][fq][64:128, qcc], ya[:, 512:1024],
                        rb[:, 512:1024],
                    )
                    if half == 1 and fq == 3:
                        # qc-outer ordering: all 4 fq done for this qc ->
                        # the qc's token blocks can be projected already
                        for tt in range(qc * 4, qc * 4 + 4):
                            proj_q.append((half, tt))
                    elif half == 0 and fq == 3 and qc == 1:
                        for tt in range(8):
                            proj_q.append((half, tt))

                def proj_chunks(item, obp):
                    half, tt = item
                    st = {}

                    def mm_chunk(fcr):
                        def c_(pool):
                            if fcr == 0:
                                st["po"] = pool.tile(
                                    [128, 1024], f32, tag="py", name="po_t")
                            for fc in (fcr, fcr + 1):
                                for n in range(2):
                                    nc.tensor.matmul(
                                        st["po"][:, n * 512:(n + 1) * 512],
                                        yn_cur[half][fc][:, tt * 128:(tt + 1) * 128],
                                        wp_sb[fc][:, n * 512:(n + 1) * 512],
                                        start=(fc == 0), stop=(fc == 3),
                                    )
                        return c_

                    def c_out(pool):
                        ob = obp.tile([128, C], f16, tag="ob")
                        nc.vector.tensor_copy(ob[:], st["po"][:])
                        nc.sync.dma_start(
                            out_d[half * 1024 + tt * 128:
                                  half * 1024 + (tt + 1) * 128, :],
                            ob[:],
                        )

                    return [mm_chunk(0), mm_chunk(2), c_out]

                def run_stream(pss, psy, units, fillers, obp=None,
                               chunked=False):
                    # scores(i+1) AND exp(i+1) are traced before y(i): the
                    # queued exp keeps ACT fed through PE bursts.
                    # chunked=False (PE-bound stream): two whole fillers at
                    # each pair-group boundary. chunked=True (ACT-paced
                    # stream): filler/proj chunk lists drain one ~1us chunk
                    # per unit so no PE burst outruns the 1-exp lookahead.
                    fq_ = list(fillers)
                    work = []

                    def pump(n=1):
                        for _ in range(n):
                            if work:
                                work.pop(0)(psy)

                    ps_i = emit_scores(pss, units[0])
                    e_i = emit_exp(units[0], ps_i)
                    for i, u in enumerate(units):
                        half, fq, qc, kt = u
                        if half not in yn_cur:
                            yn_cur[half] = [
                                ynp.tile([128, 1024], f16, tag=f"yn{fc}", name=f"yn{fc}")
                                for fc in range(4)
                            ]
                        if i + 1 < len(units):
                            ps_n = emit_scores(pss, units[i + 1])
                            e_n = emit_exp(units[i + 1], ps_n)
                        if chunked:
                            # one chunk ahead of a group-start y (the py slot
                            # frees after the previous norm's ~2.3us copies;
                            # two chunks here delayed the next scores issue
                            # and starved ACT ~2.6us at boundaries);
                            # otherwise drip every other unit
                            pump(1 if (kt == 0 or i % 2 == 0) else 0)
                        emit_y(psy, u, e_i)
                        if kt == 8 * half + 4 * qc + 3:
                            emit_norm(u)
                            if chunked:
                                if fq_:
                                    work.extend(fq_.pop(0))
                                pump(1)
                            else:
                                for _ in range(2):
                                    if fq_:
                                        for ch in fq_.pop(0):
                                            ch(psy)
                        if (obp is not None and proj_q and i % 9 == 0
                                and len(work) < 4):
                            work.extend(proj_chunks(proj_q.pop(0), obp))
                        if i + 1 < len(units):
                            e_i = e_n
                    while fq_:
                        work.extend(fq_.pop(0))
                    if obp is not None:
                        while proj_q:
                            work.extend(proj_chunks(proj_q.pop(0), obp))
                    # tail drain allocates from the SCORES pool slots (tag
                    # "s", same [128,1024] shape): those free the moment the
                    # last exp has read them, while psy's slot only frees
                    # after the final norm's DVE copies - draining from psy
                    # left the PE idle ~2.5us there, which also dropped the
                    # HAM clock to 1.2GHz for the whole projection tail
                    class _STag:
                        def tile(self, shape, dtype, tag=None, name=None):
                            return pss.tile(shape, dtype, tag="s", name=name)
                    stag = _STag()
                    while work:
                        work.pop(0)(stag)

                def make_units(half):
                    if half == 0:
                        return [(0, fq, qc, kt)
                                for fq in range(4)
                                for qc in range(2)
                                for kt in range(4 * qc + 4)]
                    # half 1: qc-outer so the v12-15 fillers (emitted at the
                    # first group boundaries) land before any kt>=12 unit
                    return [(1, fq, qc, kt)
                            for qc in range(2)
                            for fq in range(4)
                            for kt in range(8 + 4 * qc + 4)]

                units0 = make_units(0)
                units1 = make_units(1)

                # interleaved QKV work items for tokens 1024..2047.
                # q/k features ordered (0,4),(1,5),.. so half-1's fq-ordered
                # groups see their features early; v12-15 move to half-1's
                # stream (only kt>=12 units need them).
                fillers0 = []
                for f in (0, 4, 1, 5, 2, 6, 3, 7):
                    fillers0.append(qk_feature_chunks(f, x23, 1024))
                for tl in range(4):
                    fillers0.append(v_tile_chunks(tl, x23, 8 + tl, act_copy=True))
                fillers1 = [v_tile_chunks(tl, x23, 8 + tl) for tl in range(4, 8)]

                with (
                    tc.tile_pool(name="pss0", bufs=2, space="PSUM") as pss0,
                    tc.tile_pool(name="psy0", bufs=2, space="PSUM") as psy0,
                ):
                    run_stream(pss0, psy0, units0, fillers0, chunked=False)

                with (
                    tc.tile_pool(name="obp", bufs=2) as obp,
                    tc.tile_pool(name="pss1", bufs=2, space="PSUM") as pss1,
                    tc.tile_pool(name="psy1", bufs=2, space="PSUM") as psy1,
                ):
                    run_stream(pss1, psy1, units1, fillers1, obp=obp,
                               chunked=True)

    nc.compile()
    return nc


def _get_nc():
    if "nc" not in _CACHE:
        _CACHE["nc"] = _build_nc()
    return _CACHE["nc"]


def prepare_in_maps(x, W_attn, b_attn, W_proj, b_proj):
    import ml_dtypes
    x = np.asarray(x, dtype=np.float32)
    W_attn = np.asarray(W_attn, dtype=np.float32)
    b_attn = np.asarray(b_attn, dtype=np.float32)
    W_proj = np.asarray(W_proj, dtype=np.float32)

    mask = np.zeros((128, 256), np.float32)
    mask[:, 0:128] = np.triu(np.ones((128, 128), np.float32), 1)
    mask[:, 128:256] = -1e30 * np.eye(128, dtype=np.float32)
    maskb = np.ascontiguousarray(mask.astype(ml_dtypes.bfloat16))
    ones = np.ones((1, 128), np.float16)
    xts = [np.ascontiguousarray(x[b].T.astype(np.float16)) for b in range(4)]

    in_maps = []
    for c in range(8):
        b, hg = divmod(c, 2)
        s = hg * 512
        wqkv = np.ascontiguousarray(np.concatenate(
            [W_attn[:, s:s + 512],
             W_attn[:, 1024 + s:1024 + s + 512],
             W_attn[:, 2048 + s:2048 + s + 512]], axis=1).astype(np.float16))
        bqk = np.ascontiguousarray(
            np.concatenate([b_attn[s:s + 512], b_attn[1024 + s:1024 + s + 512]])
            .reshape(8, 128).T)
        bv = np.ascontiguousarray(
            b_attn[2048 + s:2048 + s + 512].reshape(1, 512).astype(np.float16))
        wproj = np.ascontiguousarray(
            W_proj[s:s + 512, :].astype(np.float16))
        in_maps.append({"xt": xts[b], "wqkv": wqkv, "bqk": bqk, "bv": bv,
                        "wproj": wproj, "ones": ones, "maskb": maskb})
    return in_maps


def kernel(x, W_attn, b_attn, W_proj, b_proj):
    from concourse.bass_utils import run_bass_kernel_spmd

    b_proj = np.asarray(b_proj, dtype=np.float32)
    nc = _get_nc()
    in_maps = prepare_in_maps(x, W_attn, b_attn, W_proj, b_proj)

    res = run_bass_kernel_spmd(nc, in_maps, core_ids=list(range(8)))
    y = np.empty((4, T, C), np.float32)
    for b in range(4):
        y[b] = (res.results[2 * b]["out"].astype(np.float32)
                + res.results[2 * b + 1]["out"].astype(np.float32) + b_proj)
    return y
